# revision 1
# baseline (speedup 1.0000x reference)
"""GNN message passing (SpMM) on 8 Trainium2 NeuronCores.

Computes out = segment_sum((X @ W)[cols] * vals, rows) for
X [50000, 128] f32, W [128, 128], 800k edges -- as
out = segment_sum(vals * X[cols]) @ W  (linearity), so the device
gathers raw X rows (bf16 table), accumulates per-destination via
one-hot matmul-scatter on the TensorEngine, and applies W once per
128-destination block.

Sharding: destinations split evenly across the 8 cores (6250 each);
edges partitioned by destination. Each core's edges are grouped by
(dest-block of 128, source-window of 25000) and sorted by source so
gather indices fit in int16 (dma_gather requirement). One SPMD
program serves all 8 cores: per-core blocks are rank-matched by edge
count to program "slots" so the per-slot tile counts (max across
cores) stay close to each core's actual counts; pad edges carry
val=0. The host un-permutes output blocks.
"""

import numpy as np
import ml_dtypes

import concourse.bacc as bacc
import concourse.bass as bass
import concourse.mybir as mybir
import concourse.tile as tile
from concourse.bass_utils import run_bass_kernel_spmd

N_NODES = 50000
N_EDGES = 800000
F = 128
NCORES = 8
NPC = N_NODES // NCORES          # 6250 destinations per core
BLK = 128
NB = (NPC + BLK - 1) // BLK      # 49 blocks (last has 106 rows)
NWIN = 2
WINBASE = N_NODES // NWIN        # 25000 (< int16 max 32767)
CB = 8                           # block-slots per gather call
CHUNKS = [list(range(s, min(s + CB, NB))) for s in range(0, NB, CB)]
NCHUNK = len(CHUNKS)

BF16 = mybir.dt.bfloat16
F32 = mybir.dt.float32
I16 = mybir.dt.int16

_CACHE = {}


def _prep(adj_rows, adj_cols, adj_vals):
    """Host-side sharding: returns (structure, per-core arrays, block perm)."""
    dst = np.asarray(adj_rows).astype(np.int64)
    src = np.asarray(adj_cols).astype(np.int64)
    val = np.asarray(adj_vals).astype(np.float32)

    core = dst // NPC
    block = (dst % NPC) // BLK
    drel = (dst % NPC) % BLK
    win = (src >= WINBASE).astype(np.int64)

    key = (core * NB + block) * NWIN + win
    cnt = np.bincount(key, minlength=NCORES * NB * NWIN).reshape(NCORES, NB, NWIN)

    # rank-match blocks across cores: slot s = core k's rank-s block (by
    # total edge count) so max_k cnt stays close to each core's count
    perm = np.argsort(-(cnt.sum(axis=2)), axis=1, kind="stable")  # [NCORES, NB]
    slot_of_block = np.empty_like(perm)
    for k in range(NCORES):
        slot_of_block[k, perm[k]] = np.arange(NB)
    cnt_slot = np.take_along_axis(cnt, perm[:, :, None], axis=1)  # [NCORES, S, W]

    t = ((cnt_slot + BLK - 1) // BLK).max(axis=0)  # [NB(slots), NWIN]
    t[:, 0] = np.maximum(t[:, 0], 1)

    # call structure: issue order c=0..NCHUNK-1, within chunk w=0,1
    calloff = np.zeros((NB, NWIN), dtype=np.int64)
    ncall = np.zeros((NCHUNK, NWIN), dtype=np.int64)
    for c, blocks in enumerate(CHUNKS):
        for w in range(NWIN):
            o = 0
            for s in blocks:
                calloff[s, w] = o
                o += t[s, w]
            ncall[c, w] = o * BLK
    idx_off = np.zeros((NCHUNK, NWIN), dtype=np.int64)
    o = 0
    for c in range(NCHUNK):
        for w in range(NWIN):
            idx_off[c, w] = o
            o += ncall[c, w] // 16
    idx_cols = int(o)

    slot_base = np.concatenate([[0], np.cumsum(t[:, 0] + t[:, 1])[:-1]])
    nt = int((t[:, 0] + t[:, 1]).sum())

    # per-edge placement
    order = np.lexsort((src, key))
    key_s = key[order]
    gstart = np.zeros(NCORES * NB * NWIN + 1, dtype=np.int64)
    np.cumsum(cnt.ravel(), out=gstart[1:])
    rank = np.arange(N_EDGES, dtype=np.int64) - gstart[key_s]

    core_s = core[order]
    block_s = block[order]
    win_s = win[order]
    drel_s = drel[order]
    src_s = src[order]
    val_s = val[order]
    slot_s = slot_of_block[core_s, block_s]
    chunk_s = slot_s // CB
    j_s = rank // BLK
    p_s = rank % BLK
    dv_col = slot_base[slot_s] + np.where(win_s == 0, j_s, t[slot_s, 0] + j_s)
    q = BLK * (calloff[slot_s, win_s] + j_s) + p_s
    icol = idx_off[chunk_s, win_s] + q // 16
    irow = q % 16

    idxbase = np.zeros((NCORES, 16, idx_cols), dtype=np.int16)
    idxbase[core_s, irow, icol] = (src_s - win_s * WINBASE).astype(np.int16)
    dmat = np.zeros((NCORES, BLK, nt), dtype=np.float32)
    dmat[core_s, p_s, dv_col] = drel_s.astype(np.float32)
    vmat = np.zeros((NCORES, BLK, nt), dtype=np.float32)
    vmat[core_s, p_s, dv_col] = val_s

    idx = np.tile(idxbase, (1, 8, 1))

    struct = dict(
        t=t, calloff=calloff, ncall=ncall, idx_off=idx_off,
        idx_cols=idx_cols, slot_base=slot_base, nt=nt,
    )
    return struct, idx, dmat, vmat, perm


def _build(struct, rep=1, gbufs=3):
    t = struct["t"]
    calloff = struct["calloff"]
    ncall = struct["ncall"]
    idx_off = struct["idx_off"]
    slot_base = struct["slot_base"]
    nt = struct["nt"]

    nc = bacc.Bacc("TRN2", debug=False, num_swdge_queues=4)
    x = nc.declare_dram_parameter("x", [N_NODES, F], BF16, isOutput=False)
    wm = nc.declare_dram_parameter("wm", [F, F], BF16, isOutput=False)
    iotam = nc.declare_dram_parameter("iotam", [BLK, BLK], BF16, isOutput=False)
    idxp = nc.declare_dram_parameter(
        "idx", [BLK, struct["idx_cols"]], I16, isOutput=False
    )
    dmatp = nc.declare_dram_parameter("dmat", [BLK, nt], F32, isOutput=False)
    vmatp = nc.declare_dram_parameter("vmat", [BLK, nt], F32, isOutput=False)
    outp = nc.declare_dram_parameter("out", [NB * BLK, F], BF16, isOutput=True)

    xw = [x[0:WINBASE, :], x[WINBASE:N_NODES, :]]

    with tile.TileContext(nc) as tc:
        with (
            tc.tile_pool(name="const", bufs=1) as constp,
            tc.tile_pool(name="g0", bufs=gbufs) as g0p,
            tc.tile_pool(name="g1", bufs=gbufs) as g1p,
            tc.tile_pool(name="st", bufs=6) as stp,
            tc.tile_pool(name="psa", bufs=3, space="PSUM") as psap,
            tc.tile_pool(name="pso", bufs=2, space="PSUM") as psop,
            tc.tile_pool(name="acct", bufs=3) as acctp,
            tc.tile_pool(name="outs", bufs=1) as outsp,
        ):
            d_t = constp.tile([BLK, nt], F32, tag="d_t")
            nc.sync.dma_start(out=d_t[:], in_=dmatp[:])
            v_t = constp.tile([BLK, nt], F32, tag="v_t")
            nc.sync.dma_start(out=v_t[:], in_=vmatp[:])
            w_t = constp.tile([F, F], BF16, tag="w_t")
            nc.sync.dma_start(out=w_t[:], in_=wm[:])
            iota_t = constp.tile([BLK, BLK], BF16, tag="iota_t")
            nc.sync.dma_start(out=iota_t[:], in_=iotam[:])
            out_stage = outsp.tile([BLK, NB, F], BF16)

            # per-call idx tiles so early gathers don't wait on one big load
            idx_tiles = {}
            for c in range(NCHUNK):
                for w in range(NWIN):
                    n = int(ncall[c, w])
                    if n == 0:
                        continue
                    io = int(idx_off[c, w])
                    it = constp.tile([BLK, n // 16], I16, tag=f"idx_{c}_{w}")
                    nc.sync.dma_start(out=it[:], in_=idxp[:, io : io + n // 16])
                    idx_tiles[(c, w)] = it

            import contextlib

            loop_ctx = (
                tc.For_i(0, rep, 1) if rep > 1 else contextlib.nullcontext()
            )
            with loop_ctx:
                qn = 0
                for c, blocks in enumerate(CHUNKS):
                    g = []
                    for w in range(NWIN):
                        n = int(ncall[c, w])
                        if n == 0:
                            g.append(None)
                            continue
                        gt = (g0p if w == 0 else g1p).tile(
                            [BLK, n // BLK, F], BF16, tag=f"g{w}"
                        )
                        nc.gpsimd.dma_gather(
                            gt[:], xw[w], idx_tiles[(c, w)][:], n, n, F,
                            single_packet=False, queue_num=qn % 4,
                        )
                        qn += 1
                        g.append(gt)
                    for s in blocks:
                        ntile_b = int(t[s, 0] + t[s, 1])
                        acc = psap.tile([F, BLK], F32, tag="acc")
                        k = 0
                        for w in range(NWIN):
                            for j in range(int(t[s, w])):
                                st = stp.tile([BLK, BLK], BF16, tag="st")
                                col = int(
                                    slot_base[s] + (j if w == 0 else t[s, 0] + j)
                                )
                                nc.vector.tensor_scalar(
                                    out=st[:],
                                    in0=iota_t[:],
                                    scalar1=d_t[:, col : col + 1],
                                    scalar2=v_t[:, col : col + 1],
                                    op0=mybir.AluOpType.is_equal,
                                    op1=mybir.AluOpType.mult,
                                )
                                gtile = g[w][:, int(calloff[s, w] + j), :]
                                nc.tensor.matmul(
                                    out=acc[:],
                                    lhsT=gtile,
                                    rhs=st[:],
                                    start=(k == 0),
                                    stop=(k == ntile_b - 1),
                                )
                                k += 1
                        acct = acctp.tile([F, BLK], BF16, tag="acct")
                        nc.scalar.copy(out=acct[:], in_=acc[:])
                        ops = psop.tile([BLK, F], F32, tag="ops")
                        nc.tensor.matmul(
                            out=ops[:], lhsT=acct[:], rhs=w_t[:], start=True,
                            stop=True,
                        )
                        nc.scalar.copy(out=out_stage[:, s, :], in_=ops[:])
                    # stream this chunk's output slice out now
                    lo, hi = blocks[0], blocks[-1] + 1
                    out_ap = outp[lo * BLK : hi * BLK, :].rearrange(
                        "(b d) o -> d b o", d=BLK
                    )
                    nc.sync.dma_start(out=out_ap, in_=out_stage[:, lo:hi, :])
    nc.compile()
    return nc


_LAST_STRUCT = None


def _in_maps_for(inputs, idx, dmat, vmat):
    xb = np.asarray(inputs["input"], dtype=np.float32).astype(ml_dtypes.bfloat16)
    wb = np.asarray(inputs["weight"], dtype=np.float32).astype(ml_dtypes.bfloat16)
    iota = np.tile(np.arange(BLK, dtype=np.float32), (BLK, 1)).astype(
        ml_dtypes.bfloat16
    )
    return [
        {"x": xb, "wm": wb, "iotam": iota, "idx": idx[k],
         "dmat": dmat[k], "vmat": vmat[k]}
        for k in range(NCORES)
    ]


def _timing_handles(inputs):
    """(nc_rep1, in_maps) for the rep-delta timing harness in test.py."""
    global _LAST_STRUCT
    struct, idx, dmat, vmat, perm = _prep(
        inputs["adj_rows"], inputs["adj_cols"], inputs["adj_vals"]
    )
    _LAST_STRUCT = struct
    ckey = (struct["idx_cols"], struct["nt"], struct["t"].tobytes(),
            struct["ncall"].tobytes())
    if ckey not in _CACHE:
        _CACHE[ckey] = _build(struct)
    return _CACHE[ckey], _in_maps_for(inputs, idx, dmat, vmat)


def _build_rep(R):
    assert _LAST_STRUCT is not None
    return _build(_LAST_STRUCT, rep=R)


def kernel(input, weight, adj_rows, adj_cols, adj_vals):
    x = np.asarray(input, dtype=np.float32)
    w = np.asarray(weight, dtype=np.float32)

    struct, idx, dmat, vmat, perm = _prep(adj_rows, adj_cols, adj_vals)

    ckey = (struct["idx_cols"], struct["nt"], struct["t"].tobytes(),
            struct["ncall"].tobytes())
    if ckey in _CACHE:
        nc = _CACHE[ckey]
    else:
        nc = _build(struct)
        _CACHE[ckey] = nc

    in_maps = _in_maps_for(
        {"input": x, "weight": w}, idx, dmat, vmat
    )
    res = run_bass_kernel_spmd(nc, in_maps, core_ids=list(range(NCORES)))

    out = np.empty((N_NODES, F), dtype=np.float32)
    for k in range(NCORES):
        r = np.asarray(res.results[k]["out"]).astype(np.float32).reshape(
            NB, BLK, F
        )
        for s in range(NB):
            b = perm[k, s]
            n = min(BLK, NPC - b * BLK)
            out[k * NPC + b * BLK : k * NPC + b * BLK + n] = r[s, :n]
    return out



# revision 2
# speedup vs baseline: 2.9360x; 2.9360x over previous
"""GNN message passing (SpMM) on 8 Trainium2 NeuronCores.

Computes out = segment_sum((X @ W)[cols] * vals, rows) for X [50000, 128]
f32, W [128, 128], 800k edges -- as out = segment_sum(vals * X[cols]) @ W
(linearity): each core gathers raw X rows (bf16) for its destination
shard's edges via per-edge dma_gather, scatter-accumulates them per
128-destination block with one-hot matmuls on the TensorEngine, and
applies W once per block.

Key performance choices (vs the v1 kernel, ~755us -> ~300us):
- One-hot scatter matrices are PRECOMPUTED on the host and STREAMED from
  HBM as contiguous bf16 (nc.sync HWDGE), not built on the DVE: DVE
  tensor_scalar bursts in 2-port mode lock GPSIMD out of the SWDGE
  descriptor rings and serialize against the gather (measured fully
  additive: 389us gather + 212us DVE -> 650us; streaming removes it).
- Gather calls are kept small (2 dest blocks x 6 source windows per
  call): large calls overflow the SWDGE descriptor ring and drop to
  ~3.5ns/descriptor; small calls run at ~1.4-2ns/descriptor.
- Source windows (int16 index requirement) are load-balanced across
  cores by borrowing lowest-src edges from window w+1 into window w so
  every core fills w to an exact multiple of 128 (legal reach: 2*winbase
  <= 32768); residual tile padding lands only in the last window.
- The per-block W matmul is software-pipelined one block behind the
  accumulation so the PE never stalls on the Activation-engine
  PSUM->SBUF copy.

Sharding: destinations split evenly across 8 cores (6250 each); edges
partitioned by destination block; one SPMD program serves all cores
(per-core block permutation rank-matches edge counts so shared tile
counts stay tight). The host un-permutes output blocks.
"""

import numpy as np
import ml_dtypes

import concourse.bacc as bacc
import concourse.mybir as mybir
import concourse.tile as tile
from concourse.bass_utils import run_bass_kernel_spmd

BF16 = mybir.dt.bfloat16
F32 = mybir.dt.float32
I16 = mybir.dt.int16

N_NODES = 50000
N_EDGES = 800000
F = 128
NCORES = 8
NPC = N_NODES // NCORES          # 6250 destinations per core
BLK = 128
NB = (NPC + BLK - 1) // BLK      # 49 blocks (last has 106 rows)

NWIN = 6
CB = 2
GBUFS = 3
STCBUFS = 3
OUT_EVERY = 4

_CACHE = {}


def _prep(adj_rows, adj_cols, adj_vals, nwin=NWIN, cb=CB):
    """Host-side edge partitioning; returns (struct, per-core arrays, perm)."""
    dst = np.asarray(adj_rows).astype(np.int64)
    src = np.asarray(adj_cols).astype(np.int64)
    val = np.asarray(adj_vals).astype(np.float32)
    winbase = (N_NODES + nwin - 1) // nwin
    assert winbase < 32768
    balance = 2 * winbase <= 32768
    chunks = [list(range(s, min(s + cb, NB))) for s in range(0, NB, cb)]
    nchunk = len(chunks)

    core = dst // NPC
    block = (dst % NPC) // BLK
    drel = (dst % NPC) % BLK
    win = src // winbase

    key = (core * NB + block) * nwin + win
    cnt = np.bincount(key, minlength=NCORES * NB * nwin).reshape(
        NCORES, NB, nwin
    )
    perm = np.argsort(-(cnt.sum(axis=2)), axis=1, kind="stable")
    slot_of_block = np.empty_like(perm)
    for k in range(NCORES):
        slot_of_block[k, perm[k]] = np.arange(NB)
    cnt_slot = np.take_along_axis(cnt, perm[:, :, None], axis=1)

    if balance:
        # borrow lowest-src edges of window w+1 into w: fill w to an exact
        # multiple of 128 per core; residual padding only in last window
        taken = np.zeros((NCORES, NB, nwin + 1), dtype=np.int64)
        t = np.zeros((NB, nwin), dtype=np.int64)
        for w in range(nwin):
            base = cnt_slot[:, :, w] - taken[:, :, w]
            t[:, w] = -(-base.max(axis=0) // BLK)
            if w < nwin - 1:
                want = BLK * t[:, w][None, :] - base
                taken[:, :, w + 1] = np.minimum(want, cnt_slot[:, :, w + 1])
    else:
        taken = np.zeros((NCORES, NB, nwin + 1), dtype=np.int64)
        t = ((cnt_slot + BLK - 1) // BLK).max(axis=0)
    t[:, 0] = np.maximum(t[:, 0], 1)

    calloff = np.zeros((NB, nwin), dtype=np.int64)
    ncall = np.zeros((nchunk, nwin), dtype=np.int64)
    for c, blocks in enumerate(chunks):
        for w in range(nwin):
            o = 0
            for s in blocks:
                calloff[s, w] = o
                o += t[s, w]
            ncall[c, w] = o * BLK
    idx_off = np.zeros((nchunk, nwin), dtype=np.int64)
    o = 0
    for c in range(nchunk):
        for w in range(nwin):
            idx_off[c, w] = o
            o += ncall[c, w] // 16
    idx_cols = int(o)

    slot_base = np.concatenate([[0], np.cumsum(t.sum(axis=1))[:-1]])
    nt = int(t.sum())

    order = np.lexsort((src, key))
    key_s = key[order]
    gstart = np.zeros(NCORES * NB * nwin + 1, dtype=np.int64)
    np.cumsum(cnt.ravel(), out=gstart[1:])
    rank = np.arange(N_EDGES, dtype=np.int64) - gstart[key_s]

    core_s = core[order]
    block_s = block[order]
    win_s = win[order]
    drel_s = drel[order]
    src_s = src[order]
    val_s = val[order]
    slot_s = slot_of_block[core_s, block_s]
    if balance:
        tk = taken[core_s, slot_s, win_s]
        borrowed = rank < tk
        wprev = np.maximum(win_s - 1, 0)
        kept_sz = (
            cnt_slot[core_s, slot_s, wprev] - taken[core_s, slot_s, wprev]
        )
        rank = np.where(borrowed, kept_sz + rank, rank - tk)
        win_s = np.where(borrowed, win_s - 1, win_s)
    chunk_s = slot_s // cb
    j_s = rank // BLK
    p_s = rank % BLK
    woff = np.zeros((NB, nwin), dtype=np.int64)
    woff[:, 1:] = np.cumsum(t[:, :-1], axis=1)
    dv_col = slot_base[slot_s] + woff[slot_s, win_s] + j_s
    q = BLK * (calloff[slot_s, win_s] + j_s) + p_s
    icol = idx_off[chunk_s, win_s] + q // 16
    irow = q % 16

    idxbase = np.zeros((NCORES, 16, idx_cols), dtype=np.int16)
    idxbase[core_s, irow, icol] = (src_s - win_s * winbase).astype(np.int16)
    idx = np.tile(idxbase, (1, 8, 1))

    # precomputed one-hot scatter matrices, streamed by the kernel:
    # stm[k][p, col*128 + d] = val * (d == drel) for edge at (p, col)
    stm = np.zeros((NCORES, BLK, nt * BLK), dtype=ml_dtypes.bfloat16)
    stm[core_s, p_s, dv_col * BLK + drel_s] = val_s.astype(ml_dtypes.bfloat16)

    struct = dict(
        t=t, calloff=calloff, ncall=ncall, idx_off=idx_off,
        idx_cols=idx_cols, slot_base=slot_base, nt=nt, nwin=nwin,
        winbase=winbase, chunks=chunks, nchunk=nchunk, woff=woff,
    )
    data = {"idx": idx, "stm": stm}
    return struct, data, perm


def _build(struct, rep=1, gbufs=GBUFS, stcbufs=STCBUFS, out_every=OUT_EVERY):
    t = struct["t"]
    calloff = struct["calloff"]
    ncall = struct["ncall"]
    idx_off = struct["idx_off"]
    slot_base = struct["slot_base"]
    nt = struct["nt"]
    nwin = struct["nwin"]
    winbase = struct["winbase"]
    chunks = struct["chunks"]
    nchunk = struct["nchunk"]

    nc = bacc.Bacc("TRN2", debug=False, num_swdge_queues=4)
    x = nc.declare_dram_parameter("x", [N_NODES, F], BF16, isOutput=False)
    wm = nc.declare_dram_parameter("wm", [F, F], BF16, isOutput=False)
    idxp = nc.declare_dram_parameter(
        "idx", [BLK, struct["idx_cols"]], I16, isOutput=False
    )
    stmp = nc.declare_dram_parameter(
        "stm", [BLK, nt * BLK], BF16, isOutput=False
    )
    outp = nc.declare_dram_parameter("out", [NB * BLK, F], BF16, isOutput=True)

    xw = [
        x[w * winbase : min((w + 1) * winbase, N_NODES), :]
        for w in range(nwin)
    ]

    from contextlib import ExitStack
    import contextlib

    with tile.TileContext(nc) as tc:
        with ExitStack() as es:
            constp = es.enter_context(tc.tile_pool(name="const", bufs=1))
            stcp = es.enter_context(tc.tile_pool(name="stc", bufs=stcbufs))
            psap = es.enter_context(
                tc.tile_pool(name="psa", bufs=3, space="PSUM")
            )
            psop = es.enter_context(
                tc.tile_pool(name="pso", bufs=2, space="PSUM")
            )
            acctp = es.enter_context(tc.tile_pool(name="acct", bufs=3))
            outsp = es.enter_context(tc.tile_pool(name="outs", bufs=2))
            gpools = [
                es.enter_context(tc.tile_pool(name=f"g{w}", bufs=gbufs))
                for w in range(nwin)
            ]

            w_t = constp.tile([F, F], BF16, tag="w_t")
            nc.sync.dma_start(out=w_t[:], in_=wm[:])

            idx_tiles = {}
            for c in range(nchunk):
                for w in range(nwin):
                    n = int(ncall[c, w])
                    if n == 0:
                        continue
                    io = int(idx_off[c, w])
                    it = constp.tile([BLK, n // 16], I16, tag=f"idx_{c}_{w}")
                    nc.sync.dma_start(
                        out=it[:], in_=idxp[:, io : io + n // 16]
                    )
                    idx_tiles[(c, w)] = it

            loop_ctx = (
                tc.For_i(0, rep, 1) if rep > 1 else contextlib.nullcontext()
            )
            with loop_ctx:
                qn = 0
                pending = None
                out_stage = None
                out_lo = out_hi = 0

                def flush_w(s_, acct_):
                    ops = psop.tile([BLK, F], F32, tag="ops")
                    nc.tensor.matmul(
                        out=ops[:], lhsT=acct_[:], rhs=w_t[:], start=True,
                        stop=True,
                    )
                    nc.scalar.copy(
                        out=out_stage[:, s_ - out_lo, :], in_=ops[:]
                    )

                for ci, blocks in enumerate(chunks):
                    if out_stage is None:
                        out_lo = blocks[0]
                        out_hi = min(out_lo + out_every * CB, NB)
                        out_stage = outsp.tile(
                            [BLK, out_hi - out_lo, F], BF16, tag="outst"
                        )
                    g = []
                    for w in range(nwin):
                        n = int(ncall[ci, w])
                        if n == 0:
                            g.append(None)
                            continue
                        gt = gpools[w].tile(
                            [BLK, n // BLK, F], BF16, tag=f"g{w}"
                        )
                        nc.gpsimd.dma_gather(
                            gt[:], xw[w], idx_tiles[(ci, w)][:], n, n, F,
                            single_packet=False, queue_num=qn % 4,
                        )
                        qn += 1
                        g.append(gt)
                    cols0 = int(slot_base[blocks[0]])
                    ntc = int(sum(int(t[s].sum()) for s in blocks))
                    stch = stcp.tile([BLK, ntc, BLK], BF16, tag="stch")
                    nc.sync.dma_start(
                        out=stch[:],
                        in_=stmp[:, cols0 * BLK : (cols0 + ntc) * BLK]
                        .rearrange("p (c d) -> p c d", d=BLK),
                    )
                    for s in blocks:
                        ntile_b = int(t[s].sum())
                        acc = psap.tile([F, BLK], F32, tag="acc")
                        k = 0
                        for w in range(nwin):
                            for j in range(int(t[s, w])):
                                col = (
                                    int(slot_base[s] + struct["woff"][s, w])
                                    + j
                                )
                                st = stch[:, col - cols0, :]
                                gtile = g[w][:, int(calloff[s, w] + j), :]
                                nc.tensor.matmul(
                                    out=acc[:],
                                    lhsT=gtile,
                                    rhs=st,
                                    start=(k == 0),
                                    stop=(k == ntile_b - 1),
                                )
                                k += 1
                        acct = acctp.tile([F, BLK], BF16, tag="acct")
                        nc.scalar.copy(out=acct[:], in_=acc[:])
                        if pending is not None:
                            flush_w(*pending)
                        pending = (s, acct)
                    if blocks[-1] + 1 >= out_hi or ci == nchunk - 1:
                        if pending is not None:
                            flush_w(*pending)
                            pending = None
                        lo, hi = out_lo, min(out_hi, NB)
                        out_ap = outp[lo * BLK : hi * BLK, :].rearrange(
                            "(b d) o -> d b o", d=BLK
                        )
                        nc.sync.dma_start(
                            out=out_ap, in_=out_stage[:, 0 : hi - lo, :]
                        )
                        out_stage = None
    nc.compile()
    return nc


def _in_maps_for(inputs, data):
    xb = np.asarray(inputs["input"], dtype=np.float32).astype(
        ml_dtypes.bfloat16
    )
    wb = np.asarray(inputs["weight"], dtype=np.float32).astype(
        ml_dtypes.bfloat16
    )
    return [
        {"x": xb, "wm": wb, "idx": data["idx"][k],
         "stm": data["stm"][k].reshape(BLK, -1)}
        for k in range(NCORES)
    ]


_LAST_STRUCT = None


def _timing_handles(inputs):
    """(nc_rep1, in_maps) for the rep-delta timing harness in test.py."""
    global _LAST_STRUCT
    struct, data, perm = _prep(
        inputs["adj_rows"], inputs["adj_cols"], inputs["adj_vals"]
    )
    _LAST_STRUCT = struct
    ckey = (struct["idx_cols"], struct["nt"], struct["t"].tobytes())
    if ckey not in _CACHE:
        _CACHE[ckey] = _build(struct)
    return _CACHE[ckey], _in_maps_for(inputs, data)


def _build_rep(R):
    assert _LAST_STRUCT is not None
    return _build(_LAST_STRUCT, rep=R)


def kernel(input, weight, adj_rows, adj_cols, adj_vals):
    x = np.asarray(input, dtype=np.float32)
    w = np.asarray(weight, dtype=np.float32)

    struct, data, perm = _prep(adj_rows, adj_cols, adj_vals)

    ckey = (struct["idx_cols"], struct["nt"], struct["t"].tobytes())
    if ckey in _CACHE:
        nc = _CACHE[ckey]
    else:
        nc = _build(struct)
        _CACHE[ckey] = nc

    in_maps = _in_maps_for({"input": x, "weight": w}, data)
    res = run_bass_kernel_spmd(nc, in_maps, core_ids=list(range(NCORES)))

    out = np.empty((N_NODES, F), dtype=np.float32)
    for k in range(NCORES):
        r = np.asarray(res.results[k]["out"]).astype(np.float32).reshape(
            NB, BLK, F
        )
        for s in range(NB):
            b = perm[k, s]
            n = min(BLK, NPC - b * BLK)
            out[k * NPC + b * BLK : k * NPC + b * BLK + n] = r[s, :n]
    return out
